# revision 16
# baseline (speedup 1.0000x reference)
"""FAConv GNN message-passing kernel for 8 Trainium2 NeuronCores.

Sharding: edges sorted by destination; core c owns destination nodes
[c*12500, (c+1)*12500).  All softmax stats are core-local -> no
collectives.  tanh bounds scores to (-1,1) so exp cannot overflow and
the reference's segment-max pass is redundant (softmax shift
invariance) -> single pass over edges.

Per core:
  Phase 0: node table tab[n] = [msg(64) | 1 | pad | a_f32] (fp16 rows,
    256B stride; a stored as raw f32 in 2 fp16 lanes) via matmuls from a
    host-pretransposed x; per-dest-node b table (b includes b_att).
  Phase 1: destinations are processed in static windows of 128
    consecutive local nodes (98/core).  Source rows are fetched with the
    dma_gather custom DMA instruction: the int16 index limit is handled
    by splitting the table into 4 equal banks of 25024 rows; each
    window's edge slots are statically partitioned per bank (capacity
    C_b tiles of 128), and each (group, bank) region is gathered by one
    multi-thousand-index dma_gather on its own SWDGE queue (4-way Q7
    parallelism).  Per 128-edge tile a one-hot S[e,n] = (colL[e]==n) is
    built with one tensor_scalar(is_equal) against an iota; one matmul
    S.T @ [ex*msg | ex] accumulates [out | denom] in PSUM (scatter-add
    as matmul).  b is expanded window-node -> edge with PE-transposed
    one-hots (S.T @ b_win), batched per half-window through PSUM.
  Finalize: out = 0.9*acc/denom + 0.1*x, written with direct DMA
    (windows are static row ranges).
"""
import sys
import os

for _p in ("/opt/trn_rl_repo", "/root/.axon_site"):
    if os.path.isdir(_p) and _p not in sys.path:
        sys.path.insert(0, _p)

import numpy as np

N_NODES = 100000
N_EDGES = 1000000
CH = 64
EPS = 0.1
NCORES = 8
NPC = N_NODES // NCORES          # owned dest nodes per core
NLOC = 12544                     # = 98 * 128 padded local dest rows
NWIN = NLOC // 128               # 98 static windows per core
G_WIN = 14                       # windows per group
NG = NWIN // G_WIN               # 7 groups
NPAD = 100096                    # = 782 * 128 padded table rows
NBANK = 4
BANKSZ = NPAD // NBANK           # 25024 rows per bank (< 32768: int16 ok)
TW = 68                          # table row elements used (of 128)

LAST = {}


def _host_prep(x, edge_index, W_att, b_att, W_msg):
    x = np.ascontiguousarray(np.asarray(x, np.float32))
    row_all = np.asarray(edge_index[0]).astype(np.int64)
    col_all = np.asarray(edge_index[1]).astype(np.int64)
    W_att = np.asarray(W_att, np.float32)
    b_att = np.asarray(b_att, np.float32)
    W_msg = np.asarray(W_msg, np.float32)

    order = np.argsort(col_all, kind="stable")
    row_s = row_all[order].astype(np.int32)
    col_s = col_all[order].astype(np.int32)
    bounds = np.searchsorted(col_s, np.arange(NCORES + 1) * NPC)

    # shared weight prep (fp16 feeds fp16 PE matmuls in phase 0)
    xT65 = np.zeros((65, NPAD), np.float16)
    xT65[:64, :N_NODES] = x.T.astype(np.float16)
    xT65[64, :N_NODES] = 1.0
    Wa = W_att[:CH, 0]
    Wb = W_att[CH:, 0]
    Wcat = np.zeros((65, TW), np.float16)
    Wcat[0:64, 0:64] = W_msg.T
    Wcat[64, 64] = 1.0
    Wcat[0:64, 66] = Wa
    Wcat[0:64, 67] = Wb
    Wcat[64, 67] = float(b_att[0])

    # per-core edge decomposition
    cores = []
    cnt_max = np.zeros(NBANK, np.int64)
    for c in range(NCORES):
        b0, b1 = bounds[c], bounds[c + 1]
        rs = row_s[b0:b1]
        cl = col_s[b0:b1] - c * NPC
        w_of = cl >> 7
        colv = (cl & 127).astype(np.float32)
        bank = rs // BANKSZ
        idx16 = (rs - bank * BANKSZ).astype(np.int16)
        cnt = np.zeros((NWIN, NBANK), np.int64)
        np.add.at(cnt, (w_of, bank), 1)
        cnt_max = np.maximum(cnt_max, cnt.max(axis=0))
        cores.append((c, rs, cl, w_of, colv, bank, idx16, cnt))

    CB = [int(-(-m // 128)) for m in cnt_max]       # tiles per window per bank
    CSUM = sum(CB)
    RB = np.concatenate([[0], np.cumsum(CB)])       # tile offset of bank b in a window
    CHG = G_WIN * CSUM                              # chunks per group

    def wrap16(flat, n_slots):
        S = n_slots // 16
        a = np.zeros((16, S), np.int16)
        n = len(flat)
        a[np.arange(n) % 16, np.arange(n) // 16] = flat
        return np.tile(a, (8, 1))

    in_maps = []
    for (c, rs, cl, w_of, colv, bank, idx16, cnt) in cores:
        ne = len(rs)
        g_of = w_of // G_WIN
        wl_of = w_of % G_WIN
        # rank of each edge within its (window, bank) run
        okey = (w_of.astype(np.int64) * NBANK + bank)
        eorder = np.argsort(okey, kind="stable")
        sk = okey[eorder]
        runstart = np.concatenate([[0], np.flatnonzero(sk[1:] != sk[:-1]) + 1])
        rank = np.arange(ne) - np.repeat(runstart, np.diff(np.concatenate([runstart, [ne]])))
        krank = np.empty(ne, np.int64)
        krank[eorder] = rank
        # group staging layout: [bank region][window][tile]
        cc_in_call = wl_of * np.take(CB, bank) + (krank // 128)
        p_of = krank % 128
        # global chunk id (for colL): g*CHG + G*RB[bank] + cc_in_call
        gchunk = g_of * CHG + G_WIN * RB[bank] + cc_in_call

        colL = np.full((128, NG * CHG), -1.0, np.float32)
        colL[p_of, gchunk] = colv
        # per-bank gather index arrays: per group a [128, S] block
        idx_blocks = {b: [] for b in range(NBANK)}
        for g in range(NG):
            for b in range(NBANK):
                n_slots = G_WIN * CB[b] * 128
                sel = (g_of == g) & (bank == b)
                j = cc_in_call[sel] * 128 + p_of[sel]
                flat = np.zeros(n_slots, np.int16)
                flat[j] = idx16[sel]
                idx_blocks[b].append(wrap16(flat, n_slots))
        bank_idx = [np.concatenate(idx_blocks[b], axis=1) for b in range(NBANK)]

        x_own = np.zeros((NLOC, CH), np.float32)
        x_own[:NPC] = x[c * NPC:(c + 1) * NPC]
        xTown = np.zeros((65, NLOC), np.float16)
        xTown[:64] = x_own.T.astype(np.float16)
        xTown[64] = 1.0

        m = {
            "xT": xT65, "Wcat": Wcat, "xTown": xTown, "x_own": x_own,
            "colL": colL,
        }
        for b in range(NBANK):
            m[f"rix{b}"] = bank_idx[b]
        in_maps.append(m)
    return in_maps, CB, float(b_att[0])


def build_program(CB, ncores=NCORES):
    import concourse.bacc as bacc
    import concourse.mybir as mybir
    import concourse.tile as tile
    from concourse.bass import ts

    f32 = mybir.dt.float32
    fp16 = mybir.dt.float16
    i16 = mybir.dt.int16
    i32 = mybir.dt.int32
    AF = mybir.ActivationFunctionType
    ALU = mybir.AluOpType

    CSUM = sum(CB)
    RB = [0]
    for b in range(NBANK):
        RB.append(RB[-1] + CB[b])
    CHG = G_WIN * CSUM
    HALF = (CSUM + 1) // 2

    # Tile's DMASW semaphore rotation is SWDGE-queue-blind; with 4 queues the
    # sim/HW shadow-sem bookkeeping requires each DMASW lane to serve one
    # queue.  Patch the lane assignment: queue q uses lanes {2q, 2q+1}.
    import concourse.tile_sem_assignment as tsa
    from concourse.tile_scheduler import DMAInst as _DMAInst

    if not getattr(tsa.TileClockTick, "_q_aware_patch", False):
        _orig_assign_tick = tsa.TileClockTick._assign_tick

        def _assign_tick_qaware(self, inst):
            q = getattr(inst, "queue_num", None)
            if (q is not None and inst.engine == mybir.EngineType.Pool
                    and isinstance(inst, _DMAInst)):
                if not hasattr(self, "_qrr"):
                    self._qrr = [0, 0, 0, 0]
                save = self.next_sw_dma_idx
                self.next_sw_dma_idx = 2 * q + (self._qrr[q] & 1)
                self._qrr[q] += 1
                _orig_assign_tick(self, inst)
                self.next_sw_dma_idx = save
                return
            return _orig_assign_tick(self, inst)

        tsa.TileClockTick._assign_tick = _assign_tick_qaware
        tsa.TileClockTick._q_aware_patch = True

    nc = bacc.Bacc("TRN2", target_bir_lowering=False, debug=False,
                   num_devices=ncores, num_swdge_queues=4)

    def raw_dma_gather(out_ap, in_ap, idxs_ap, num_idxs, elem_size, elem_step,
                       queue_num):
        g = nc.gpsimd
        stride_bytes = elem_step * mybir.dt.size(in_ap.dtype)
        assert stride_bytes % 256 == 0
        _in_ap = g.lower_ap_dma(in_ap, for_custom_bir_dma=True)
        _idxs_ap = g.lower_ap(idxs_ap)
        _out_ap = g.lower_ap(out_ap)
        return g.add_instruction(
            mybir.InstDMAGatherAnt(
                name=g.bass.get_next_instruction_name(),
                ins=[*_in_ap, _idxs_ap, g.lower_val_access(g.to_reg(num_idxs))],
                outs=[_out_ap],
                transpose=False, num_idxs=num_idxs, elem_size=elem_size,
                stride_bytes_256=stride_bytes // 256, gen_mode=0,
                single_packet=False, queue_num=queue_num,
                sbuf_tokens_per_rank=0, sbuf_free_dim_per_rank=0,
                sbuf_free_dim_pad_per_rank=0, sbuf_byte_offset=0,
            )
        )

    xT_d = nc.dram_tensor("xT", [65, NPAD], fp16, kind="ExternalInput")
    wcat_d = nc.dram_tensor("Wcat", [65, TW], fp16, kind="ExternalInput")
    xTown_d = nc.dram_tensor("xTown", [65, NLOC], fp16, kind="ExternalInput")
    xown_d = nc.dram_tensor("x_own", [NLOC, CH], f32, kind="ExternalInput")
    colL_d = nc.dram_tensor("colL", [128, NG * CHG], f32, kind="ExternalInput")
    rix_d = []
    for b in range(NBANK):
        S = NG * G_WIN * CB[b] * 8
        rix_d.append(nc.dram_tensor(f"rix{b}", [128, S], i16,
                                    kind="ExternalInput"))
    out_d = nc.dram_tensor("out", [NLOC, CH], f32, kind="ExternalOutput")
    tab_d = nc.dram_tensor("tab", [NPAD, 128], fp16)
    bown_d = nc.dram_tensor("b_own", [NLOC, 1], f32)

    with tile.TileContext(nc) as tc:
        with (
            tc.tile_pool(name="const", bufs=1) as cpool,
            tc.tile_pool(name="p0", bufs=3) as p0pool,
            tc.tile_pool(name="gin", bufs=2) as ginpool,
            tc.tile_pool(name="work", bufs=4) as wpool,
            tc.tile_pool(name="ps0", bufs=2, space="PSUM") as ps0pool,
            tc.tile_pool(name="psA", bufs=2, space="PSUM") as psApool,
            tc.tile_pool(name="psT", bufs=2, space="PSUM") as psTpool,
        ):
            wc_sb = cpool.tile([65, TW], fp16)
            nc.sync.dma_start(out=wc_sb[:], in_=wcat_d[:, :])
            iota_i = cpool.tile([128, 128], i32)
            nc.gpsimd.iota(iota_i[:], pattern=[[1, 128]], base=0,
                           channel_multiplier=0)
            iota_g = cpool.tile([128, 128], fp16)
            nc.vector.tensor_copy(out=iota_g[:], in_=iota_i[:])
            ident = cpool.tile([128, 128], fp16)
            from concourse.masks import make_identity
            make_identity(nc, ident[:])

            # ---- phase 0a: global node table (fp16, batched DMA) ----
            def phase0a_block(i0, ncols, tagsfx):
                xt_t = p0pool.tile([65, ncols], fp16, tag="xt" + tagsfx)
                nc.sync.dma_start(out=xt_t[:], in_=xT_d[:, i0:i0 + ncols])
                nchunk = ncols // 128
                ot8 = p0pool.tile([128, nchunk, TW], fp16, tag="ot" + tagsfx)
                for j in range(nchunk):
                    ps0 = ps0pool.tile([128, TW], f32, tag="ps0")
                    nc.tensor.matmul(ps0[:], lhsT=xt_t[:, ts(j, 128)],
                                     rhs=wc_sb[:], start=True, stop=True)
                    nc.any.tensor_copy(out=ot8[:, j, :], in_=ps0[:])
                nc.scalar.dma_start(
                    out=tab_d[i0:i0 + ncols, 0:TW].rearrange(
                        "(j p) c -> p j c", p=128),
                    in_=ot8[:])

            for i in range(NPAD // 1024):
                phase0a_block(i * 1024, 1024, "")
            if NPAD % 1024:
                phase0a_block((NPAD // 1024) * 1024, NPAD % 1024, "r")

            # ---- phase 0b: per-dest b table ----
            def phase0b_block(i0, ncols, tagsfx):
                xo_t = p0pool.tile([65, ncols], fp16, tag="xo" + tagsfx)
                nc.sync.dma_start(out=xo_t[:], in_=xTown_d[:, i0:i0 + ncols])
                nchunk = ncols // 128
                bt8 = p0pool.tile([128, nchunk, 1], f32, tag="bt" + tagsfx)
                for j in range(nchunk):
                    psb = ps0pool.tile([128, TW], f32, tag="ps0")
                    nc.tensor.matmul(psb[:], lhsT=xo_t[:, ts(j, 128)],
                                     rhs=wc_sb[:], start=True, stop=True)
                    nc.any.tensor_copy(out=bt8[:, j, :], in_=psb[:, 67:68])
                nc.scalar.dma_start(
                    out=bown_d[i0:i0 + ncols, :].rearrange(
                        "(j p) c -> p j c", p=128),
                    in_=bt8[:])

            for i in range(NLOC // 1024):
                phase0b_block(i * 1024, 1024, "")
            if NLOC % 1024:
                phase0b_block((NLOC // 1024) * 1024, NLOC % 1024, "r")

            # ---- phase 1 ----
            for g in range(NG):
                colL_t = ginpool.tile([128, CHG], f32, tag="colL")
                nc.scalar.dma_start(out=colL_t[:],
                                  in_=colL_d[:, ts(g, CHG)])
                Gbs = []
                for b in range(NBANK):
                    S = G_WIN * CB[b] * 8
                    rt = ginpool.tile([128, S], i16, tag=f"rix{b}")
                    nc.sync.dma_start(out=rt[:], in_=rix_d[b][:, ts(g, S)])
                    Gb_b = ginpool.tile([128, G_WIN * CB[b], TW], fp16,
                                        tag=f"Gb{b}")
                    # keep each call's descriptor footprint under the SWDGE
                    # ring size (~<=4096 idx) so calls overlap across queues
                    nh = G_WIN * CB[b] // 2
                    for h in range(2):
                        raw_dma_gather(
                            Gb_b[:, h * nh:(h + 1) * nh, :],
                            tab_d[b * BANKSZ:(b + 1) * BANKSZ, 0:TW],
                            rt[:, h * nh * 8:(h + 1) * nh * 8],
                            nh * 128, TW, 128, queue_num=b)
                    Gbs.append(Gb_b)
                # group-level direct loads
                bw_t = ginpool.tile([128, G_WIN], f32, tag="bw")
                nc.scalar.dma_start(
                    out=bw_t[:],
                    in_=bown_d[ts(g, G_WIN * 128), :].rearrange(
                        "(j p) one -> p (j one)", p=128))
                xw_t = ginpool.tile([128, G_WIN, CH], f32, tag="xw")
                nc.scalar.dma_start(
                    out=xw_t[:],
                    in_=xown_d[ts(g, G_WIN * 128), :].rearrange(
                        "(j p) c -> p j c", p=128))
                outb = ginpool.tile([128, G_WIN, CH], f32, tag="outb")



                for wl in range(G_WIN):
                    chunks = [(b, wl * CB[b] + t)
                              for b in range(NBANK) for t in range(CB[b])]
                    bwin16 = wpool.tile([128, 1], fp16, tag="bwin16")
                    nc.vector.tensor_copy(out=bwin16[:], in_=bw_t[:, wl:wl + 1])

                    sts = []
                    for t, (bb, cl) in enumerate(chunks):
                        gcc = G_WIN * RB[bb] + cl
                        st = wpool.tile([128, 128], fp16, tag=f"st{t}")
                        nc.vector.tensor_scalar(
                            out=st[:], in0=iota_g[:],
                            scalar1=colL_t[:, gcc:gcc + 1], scalar2=None,
                            op0=ALU.is_equal)
                        sts.append(st)

                    # b expansion: per half-window transpose batch
                    psA = psApool.tile([128, 80], f32, tag="psA")
                    stT = wpool.tile([128, CSUM * 128], fp16, tag="stT")
                    for h in range(2):
                        lo = h * HALF
                        hi = min(CSUM, lo + HALF)
                        if lo >= hi:
                            continue
                        psT = psTpool.tile([128, HALF * 128], fp16, tag="psT")
                        for t in range(lo, hi):
                            nc.tensor.transpose(
                                out=psT[:, ts(t - lo, 128)], in_=sts[t][:],
                                identity=ident[:])
                        nc.any.tensor_copy(
                            out=stT[:, lo * 128:hi * 128],
                            in_=psT[:, 0:(hi - lo) * 128])
                    for t in range(CSUM):
                        nc.tensor.matmul(
                            psA[:, 65 + t:66 + t], lhsT=stT[:, ts(t, 128)],
                            rhs=bwin16[:], start=True, stop=True)

                    bch = wpool.tile([128, CSUM], fp16, tag="bch")
                    nc.vector.tensor_copy(out=bch[:], in_=psA[:, 65:65 + CSUM])
                    sin_t = wpool.tile([128, CSUM], fp16, tag="sin")
                    # a values: chunks are [G_WIN*RB[b]+wl*CB[b] .. +CB[b]) per bank
                    for b in range(NBANK):
                        nc.vector.tensor_tensor(
                            out=sin_t[:, RB[b]:RB[b + 1]],
                            in0=Gbs[b][:, wl * CB[b]:(wl + 1) * CB[b], 66],
                            in1=bch[:, RB[b]:RB[b + 1]],
                            op=ALU.add)
                    sc_t = wpool.tile([128, CSUM], fp16, tag="sc")
                    nc.scalar.activation(sc_t[:], sin_t[:], AF.Tanh)
                    ex_t = wpool.tile([128, CSUM], fp16, tag="ex")
                    nc.scalar.activation(ex_t[:], sc_t[:], AF.Exp)

                    for t, (bb, cl) in enumerate(chunks):
                        mt = wpool.tile([128, 65], fp16, tag="mt")
                        nc.vector.tensor_tensor(
                            out=mt[:], in0=Gbs[bb][:, cl, 0:65],
                            in1=ex_t[:, t:t + 1].to_broadcast([128, 65]),
                            op=ALU.mult)
                        nc.tensor.matmul(psA[:, 0:65], lhsT=sts[t][:],
                                         rhs=mt[:], start=(t == 0),
                                         stop=(t == CSUM - 1))

                    dn = wpool.tile([128, 1], f32, tag="dn")
                    nc.vector.tensor_scalar(out=dn[:], in0=psA[:, 64:65],
                                            scalar1=1e-30, scalar2=None,
                                            op0=ALU.max)
                    inv = wpool.tile([128, 1], f32, tag="inv")
                    nc.vector.reciprocal(inv[:], dn[:])
                    nc.vector.tensor_scalar(
                        out=outb[:, wl, :], in0=psA[:, 0:64],
                        scalar1=inv[:], scalar2=(1.0 - EPS),
                        op0=ALU.mult, op1=ALU.mult)
                xeb = ginpool.tile([128, G_WIN, CH], f32, tag="xeb")
                nc.vector.tensor_scalar_mul(
                    xeb[:].rearrange("p a b -> p (a b)"),
                    xw_t[:].rearrange("p a b -> p (a b)"), EPS)
                nc.vector.tensor_tensor(
                    out=outb[:].rearrange("p a b -> p (a b)"),
                    in0=outb[:].rearrange("p a b -> p (a b)"),
                    in1=xeb[:].rearrange("p a b -> p (a b)"), op=ALU.add)
                nc.scalar.dma_start(
                    out=out_d[ts(g, G_WIN * 128), :].rearrange(
                        "(j p) c -> p j c", p=128),
                    in_=outb[:])
    nc.compile()
    return nc


def kernel(x, edge_index, W_att, b_att, W_msg, _trace=False):
    from concourse.bass_utils import run_bass_kernel_spmd

    in_maps, CB, _ = _host_prep(x, edge_index, W_att, b_att, W_msg)
    nc = build_program(CB)
    res = run_bass_kernel_spmd(nc, in_maps, list(range(NCORES)), trace=_trace)
    LAST["res"] = res
    LAST["CB"] = CB
    out = np.concatenate([res.results[c]["out"][:NPC] for c in range(NCORES)],
                         axis=0)
    return np.ascontiguousarray(out, dtype=np.float32)


# revision 17
# speedup vs baseline: 1.1980x; 1.1980x over previous
"""FAConv GNN message-passing kernel for 8 Trainium2 NeuronCores.

Sharding: edges sorted by destination; core c owns destination nodes
[c*12500, (c+1)*12500).  All softmax stats are core-local -> no
collectives.  tanh bounds scores to (-1,1) so exp cannot overflow and
the reference's segment-max pass is redundant (softmax shift
invariance) -> single pass over edges.

Per core:
  Phase 0: node table tab[n] = [msg(64) | 1 | pad | a_f32] (fp16 rows,
    256B stride; a stored as raw f32 in 2 fp16 lanes) via matmuls from a
    host-pretransposed x; per-dest-node b table (b includes b_att).
  Phase 1: destinations are processed in static windows of 128
    consecutive local nodes (98/core).  Source rows are fetched with the
    dma_gather custom DMA instruction: the int16 index limit is handled
    by splitting the table into 4 equal banks of 25024 rows; each
    window's edge slots are statically partitioned per bank (capacity
    C_b tiles of 128), and each (group, bank) region is gathered by one
    multi-thousand-index dma_gather on its own SWDGE queue (4-way Q7
    parallelism).  Per 128-edge tile a one-hot S[e,n] = (colL[e]==n) is
    built with one tensor_scalar(is_equal) against an iota; one matmul
    S.T @ [ex*msg | ex] accumulates [out | denom] in PSUM (scatter-add
    as matmul).  b is expanded window-node -> edge with PE-transposed
    one-hots (S.T @ b_win), batched per half-window through PSUM.
  Finalize: out = 0.9*acc/denom + 0.1*x, written with direct DMA
    (windows are static row ranges).
"""
import sys
import os

for _p in ("/opt/trn_rl_repo", "/root/.axon_site"):
    if os.path.isdir(_p) and _p not in sys.path:
        sys.path.insert(0, _p)

import numpy as np

N_NODES = 100000
N_EDGES = 1000000
CH = 64
EPS = 0.1
NCORES = 8
NPC = N_NODES // NCORES          # owned dest nodes per core
NLOC = 12544                     # = 98 * 128 padded local dest rows
NWIN = NLOC // 128               # 98 static windows per core
G_WIN = 14                       # windows per group
NG = NWIN // G_WIN               # 7 groups
NPAD = 100096                    # = 782 * 128 padded table rows
NBANK = 4
BANKSZ = NPAD // NBANK           # 25024 rows per bank (< 32768: int16 ok)
TW = 68                          # table row elements used (of 128)

LAST = {}


def _host_prep(x, edge_index, W_att, b_att, W_msg):
    x = np.ascontiguousarray(np.asarray(x, np.float32))
    row_all = np.asarray(edge_index[0]).astype(np.int64)
    col_all = np.asarray(edge_index[1]).astype(np.int64)
    W_att = np.asarray(W_att, np.float32)
    b_att = np.asarray(b_att, np.float32)
    W_msg = np.asarray(W_msg, np.float32)

    order = np.argsort(col_all, kind="stable")
    row_s = row_all[order].astype(np.int32)
    col_s = col_all[order].astype(np.int32)
    bounds = np.searchsorted(col_s, np.arange(NCORES + 1) * NPC)

    # shared weight prep (fp16 feeds fp16 PE matmuls in phase 0)
    xT65 = np.zeros((65, NPAD), np.float16)
    xT65[:64, :N_NODES] = x.T.astype(np.float16)
    xT65[64, :N_NODES] = 1.0
    Wa = W_att[:CH, 0]
    Wb = W_att[CH:, 0]
    Wcat = np.zeros((65, TW), np.float16)
    Wcat[0:64, 0:64] = W_msg.T
    Wcat[64, 64] = 1.0
    Wcat[0:64, 66] = Wa
    Wcat[0:64, 67] = Wb
    Wcat[64, 67] = float(b_att[0])

    # per-core edge decomposition
    cores = []
    cnt_max = np.zeros(NBANK, np.int64)
    for c in range(NCORES):
        b0, b1 = bounds[c], bounds[c + 1]
        rs = row_s[b0:b1]
        cl = col_s[b0:b1] - c * NPC
        w_of = cl >> 7
        colv = (cl & 127).astype(np.float32)
        bank = rs // BANKSZ
        idx16 = (rs - bank * BANKSZ).astype(np.int16)
        cnt = np.zeros((NWIN, NBANK), np.int64)
        np.add.at(cnt, (w_of, bank), 1)
        cnt_max = np.maximum(cnt_max, cnt.max(axis=0))
        cores.append((c, rs, cl, w_of, colv, bank, idx16, cnt))

    CB = [int(-(-m // 128)) for m in cnt_max]       # tiles per window per bank
    CSUM = sum(CB)
    RB = np.concatenate([[0], np.cumsum(CB)])       # tile offset of bank b in a window
    CHG = G_WIN * CSUM                              # chunks per group

    def wrap16(flat, n_slots):
        S = n_slots // 16
        a = np.zeros((16, S), np.int16)
        n = len(flat)
        a[np.arange(n) % 16, np.arange(n) // 16] = flat
        return np.tile(a, (8, 1))

    in_maps = []
    for (c, rs, cl, w_of, colv, bank, idx16, cnt) in cores:
        ne = len(rs)
        g_of = w_of // G_WIN
        wl_of = w_of % G_WIN
        # rank of each edge within its (window, bank) run
        okey = (w_of.astype(np.int64) * NBANK + bank)
        eorder = np.argsort(okey, kind="stable")
        sk = okey[eorder]
        runstart = np.concatenate([[0], np.flatnonzero(sk[1:] != sk[:-1]) + 1])
        rank = np.arange(ne) - np.repeat(runstart, np.diff(np.concatenate([runstart, [ne]])))
        krank = np.empty(ne, np.int64)
        krank[eorder] = rank
        # group staging layout: [bank region][window][tile]
        cc_in_call = wl_of * np.take(CB, bank) + (krank // 128)
        p_of = krank % 128
        # global chunk id (for colL): g*CHG + G*RB[bank] + cc_in_call
        gchunk = g_of * CHG + G_WIN * RB[bank] + cc_in_call

        colL = np.full((128, NG * CHG), -1.0, np.float32)
        colL[p_of, gchunk] = colv
        # per-bank gather index arrays: per group a [128, S] block
        idx_blocks = {b: [] for b in range(NBANK)}
        for g in range(NG):
            for b in range(NBANK):
                n_slots = G_WIN * CB[b] * 128
                sel = (g_of == g) & (bank == b)
                j = cc_in_call[sel] * 128 + p_of[sel]
                flat = np.zeros(n_slots, np.int16)
                flat[j] = idx16[sel]
                idx_blocks[b].append(wrap16(flat, n_slots))
        bank_idx = [np.concatenate(idx_blocks[b], axis=1) for b in range(NBANK)]

        x_own = np.zeros((NLOC, CH), np.float32)
        x_own[:NPC] = x[c * NPC:(c + 1) * NPC]
        xTown = np.zeros((65, NLOC), np.float16)
        xTown[:64] = x_own.T.astype(np.float16)
        xTown[64] = 1.0

        m = {
            "xT": xT65, "Wcat": Wcat, "xTown": xTown, "x_own": x_own,
            "colL": colL,
        }
        for b in range(NBANK):
            m[f"rix{b}"] = bank_idx[b]
        in_maps.append(m)
    return in_maps, CB, float(b_att[0])


def build_program(CB, ncores=NCORES):
    import concourse.bacc as bacc
    import concourse.mybir as mybir
    import concourse.tile as tile
    from concourse.bass import ts

    f32 = mybir.dt.float32
    fp16 = mybir.dt.float16
    i16 = mybir.dt.int16
    i32 = mybir.dt.int32
    AF = mybir.ActivationFunctionType
    ALU = mybir.AluOpType

    CSUM = sum(CB)
    RB = [0]
    for b in range(NBANK):
        RB.append(RB[-1] + CB[b])
    CHG = G_WIN * CSUM
    HALF = (CSUM + 1) // 2

    # Tile's DMASW semaphore rotation is SWDGE-queue-blind; with 4 queues the
    # sim/HW shadow-sem bookkeeping requires each DMASW lane to serve one
    # queue.  Patch the lane assignment: queue q uses lanes {2q, 2q+1}.
    import concourse.tile_sem_assignment as tsa
    from concourse.tile_scheduler import DMAInst as _DMAInst

    if not getattr(tsa.TileClockTick, "_q_aware_patch", False):
        _orig_assign_tick = tsa.TileClockTick._assign_tick

        def _assign_tick_qaware(self, inst):
            q = getattr(inst, "queue_num", None)
            if (q is not None and inst.engine == mybir.EngineType.Pool
                    and isinstance(inst, _DMAInst)):
                if not hasattr(self, "_qrr"):
                    self._qrr = [0, 0, 0, 0]
                save = self.next_sw_dma_idx
                self.next_sw_dma_idx = 2 * q + (self._qrr[q] & 1)
                self._qrr[q] += 1
                _orig_assign_tick(self, inst)
                self.next_sw_dma_idx = save
                return
            return _orig_assign_tick(self, inst)

        tsa.TileClockTick._assign_tick = _assign_tick_qaware
        tsa.TileClockTick._q_aware_patch = True

    nc = bacc.Bacc("TRN2", target_bir_lowering=False, debug=False,
                   num_devices=ncores, num_swdge_queues=4)

    def raw_dma_gather(out_ap, in_ap, idxs_ap, num_idxs, elem_size, elem_step,
                       queue_num):
        g = nc.gpsimd
        stride_bytes = elem_step * mybir.dt.size(in_ap.dtype)
        assert stride_bytes % 256 == 0
        _in_ap = g.lower_ap_dma(in_ap, for_custom_bir_dma=True)
        _idxs_ap = g.lower_ap(idxs_ap)
        _out_ap = g.lower_ap(out_ap)
        return g.add_instruction(
            mybir.InstDMAGatherAnt(
                name=g.bass.get_next_instruction_name(),
                ins=[*_in_ap, _idxs_ap, g.lower_val_access(g.to_reg(num_idxs))],
                outs=[_out_ap],
                transpose=False, num_idxs=num_idxs, elem_size=elem_size,
                stride_bytes_256=stride_bytes // 256, gen_mode=0,
                single_packet=False, queue_num=queue_num,
                sbuf_tokens_per_rank=0, sbuf_free_dim_per_rank=0,
                sbuf_free_dim_pad_per_rank=0, sbuf_byte_offset=0,
            )
        )

    xT_d = nc.dram_tensor("xT", [65, NPAD], fp16, kind="ExternalInput")
    wcat_d = nc.dram_tensor("Wcat", [65, TW], fp16, kind="ExternalInput")
    xTown_d = nc.dram_tensor("xTown", [65, NLOC], fp16, kind="ExternalInput")
    xown_d = nc.dram_tensor("x_own", [NLOC, CH], f32, kind="ExternalInput")
    colL_d = nc.dram_tensor("colL", [128, NG * CHG], f32, kind="ExternalInput")
    rix_d = []
    for b in range(NBANK):
        S = NG * G_WIN * CB[b] * 8
        rix_d.append(nc.dram_tensor(f"rix{b}", [128, S], i16,
                                    kind="ExternalInput"))
    out_d = nc.dram_tensor("out", [NLOC, CH], f32, kind="ExternalOutput")
    tab_d = nc.dram_tensor("tab", [NPAD, 128], fp16)
    bown_d = nc.dram_tensor("b_own", [NLOC, 1], f32)

    with tile.TileContext(nc) as tc:
        with (
            tc.tile_pool(name="const", bufs=1) as cpool,
            tc.tile_pool(name="p0", bufs=3) as p0pool,
            tc.tile_pool(name="gin", bufs=2) as ginpool,
            tc.tile_pool(name="work", bufs=4) as wpool,
            tc.tile_pool(name="ps0", bufs=2, space="PSUM") as ps0pool,
            tc.tile_pool(name="psA", bufs=2, space="PSUM") as psApool,
            tc.tile_pool(name="psT", bufs=2, space="PSUM") as psTpool,
        ):
            wc_sb = cpool.tile([65, TW], fp16)
            nc.sync.dma_start(out=wc_sb[:], in_=wcat_d[:, :])
            iota_i = cpool.tile([128, 128], i32)
            nc.gpsimd.iota(iota_i[:], pattern=[[1, 128]], base=0,
                           channel_multiplier=0)
            iota_g = cpool.tile([128, 128], fp16)
            nc.vector.tensor_copy(out=iota_g[:], in_=iota_i[:])
            ident = cpool.tile([128, 128], fp16)
            from concourse.masks import make_identity
            make_identity(nc, ident[:])

            # ---- phase 0a: global node table (fp16, batched DMA) ----
            def phase0a_block(i0, ncols, tagsfx):
                xt_t = p0pool.tile([65, ncols], fp16, tag="xt" + tagsfx)
                nc.sync.dma_start(out=xt_t[:], in_=xT_d[:, i0:i0 + ncols])
                nchunk = ncols // 128
                ot8 = p0pool.tile([128, nchunk, TW], fp16, tag="ot" + tagsfx)
                for j in range(nchunk):
                    ps0 = ps0pool.tile([128, TW], f32, tag="ps0")
                    nc.tensor.matmul(ps0[:], lhsT=xt_t[:, ts(j, 128)],
                                     rhs=wc_sb[:], start=True, stop=True)
                    nc.any.tensor_copy(out=ot8[:, j, :], in_=ps0[:])
                nc.scalar.dma_start(
                    out=tab_d[i0:i0 + ncols, 0:TW].rearrange(
                        "(j p) c -> p j c", p=128),
                    in_=ot8[:])

            for i in range(NPAD // 1024):
                phase0a_block(i * 1024, 1024, "")
            if NPAD % 1024:
                phase0a_block((NPAD // 1024) * 1024, NPAD % 1024, "r")

            # ---- phase 0b: per-dest b table ----
            def phase0b_block(i0, ncols, tagsfx):
                xo_t = p0pool.tile([65, ncols], fp16, tag="xo" + tagsfx)
                nc.sync.dma_start(out=xo_t[:], in_=xTown_d[:, i0:i0 + ncols])
                nchunk = ncols // 128
                bt8 = p0pool.tile([128, nchunk, 1], f32, tag="bt" + tagsfx)
                for j in range(nchunk):
                    psb = ps0pool.tile([128, TW], f32, tag="ps0")
                    nc.tensor.matmul(psb[:], lhsT=xo_t[:, ts(j, 128)],
                                     rhs=wc_sb[:], start=True, stop=True)
                    nc.any.tensor_copy(out=bt8[:, j, :], in_=psb[:, 67:68])
                nc.scalar.dma_start(
                    out=bown_d[i0:i0 + ncols, :].rearrange(
                        "(j p) c -> p j c", p=128),
                    in_=bt8[:])

            for i in range(NLOC // 1024):
                phase0b_block(i * 1024, 1024, "")
            if NLOC % 1024:
                phase0b_block((NLOC // 1024) * 1024, NLOC % 1024, "r")

            # ---- phase 1 ----
            for g in range(NG):
                colL_t = ginpool.tile([128, CHG], f32, tag="colL")
                nc.scalar.dma_start(out=colL_t[:],
                                  in_=colL_d[:, ts(g, CHG)])
                Gbs = []
                for b in range(NBANK):
                    S = G_WIN * CB[b] * 8
                    rt = ginpool.tile([128, S], i16, tag=f"rix{b}")
                    nc.sync.dma_start(out=rt[:], in_=rix_d[b][:, ts(g, S)])
                    Gb_b = ginpool.tile([128, G_WIN * CB[b], TW], fp16,
                                        tag=f"Gb{b}")
                    # keep each call's descriptor footprint under the SWDGE
                    # ring size (~<=4096 idx) so calls overlap across queues
                    nh = G_WIN * CB[b] // 2
                    for h in range(2):
                        raw_dma_gather(
                            Gb_b[:, h * nh:(h + 1) * nh, :],
                            tab_d[b * BANKSZ:(b + 1) * BANKSZ, 0:TW],
                            rt[:, h * nh * 8:(h + 1) * nh * 8],
                            nh * 128, TW, 128, queue_num=(2 * b + h) % 4)
                    Gbs.append(Gb_b)
                # group-level direct loads
                bw_t = ginpool.tile([128, G_WIN], f32, tag="bw")
                nc.scalar.dma_start(
                    out=bw_t[:],
                    in_=bown_d[ts(g, G_WIN * 128), :].rearrange(
                        "(j p) one -> p (j one)", p=128))
                xw_t = ginpool.tile([128, G_WIN, CH], f32, tag="xw")
                nc.scalar.dma_start(
                    out=xw_t[:],
                    in_=xown_d[ts(g, G_WIN * 128), :].rearrange(
                        "(j p) c -> p j c", p=128))
                outb = ginpool.tile([128, G_WIN, CH], f32, tag="outb")



                for wl in range(G_WIN):
                    chunks = [(b, wl * CB[b] + t)
                              for b in range(NBANK) for t in range(CB[b])]
                    bwin16 = wpool.tile([128, 1], fp16, tag="bwin16")
                    nc.vector.tensor_copy(out=bwin16[:], in_=bw_t[:, wl:wl + 1])

                    sts = []
                    for t, (bb, cl) in enumerate(chunks):
                        gcc = G_WIN * RB[bb] + cl
                        st = wpool.tile([128, 128], fp16, tag=f"st{t}")
                        nc.vector.tensor_scalar(
                            out=st[:], in0=iota_g[:],
                            scalar1=colL_t[:, gcc:gcc + 1], scalar2=None,
                            op0=ALU.is_equal)
                        sts.append(st)

                    # b expansion: per half-window transpose batch
                    psA = psApool.tile([128, 80], f32, tag="psA")
                    stT = wpool.tile([128, CSUM * 128], fp16, tag="stT")
                    for h in range(2):
                        lo = h * HALF
                        hi = min(CSUM, lo + HALF)
                        if lo >= hi:
                            continue
                        psT = psTpool.tile([128, HALF * 128], fp16, tag="psT")
                        for t in range(lo, hi):
                            nc.tensor.transpose(
                                out=psT[:, ts(t - lo, 128)], in_=sts[t][:],
                                identity=ident[:])
                        nc.any.tensor_copy(
                            out=stT[:, lo * 128:hi * 128],
                            in_=psT[:, 0:(hi - lo) * 128])
                    for t in range(CSUM):
                        nc.tensor.matmul(
                            psA[:, 65 + t:66 + t], lhsT=stT[:, ts(t, 128)],
                            rhs=bwin16[:], start=True, stop=True)

                    bch = wpool.tile([128, CSUM], fp16, tag="bch")
                    nc.vector.tensor_copy(out=bch[:], in_=psA[:, 65:65 + CSUM])
                    sin_t = wpool.tile([128, CSUM], fp16, tag="sin")
                    # a values: chunks are [G_WIN*RB[b]+wl*CB[b] .. +CB[b]) per bank
                    for b in range(NBANK):
                        nc.vector.tensor_tensor(
                            out=sin_t[:, RB[b]:RB[b + 1]],
                            in0=Gbs[b][:, wl * CB[b]:(wl + 1) * CB[b], 66],
                            in1=bch[:, RB[b]:RB[b + 1]],
                            op=ALU.add)
                    sc_t = wpool.tile([128, CSUM], fp16, tag="sc")
                    nc.scalar.activation(sc_t[:], sin_t[:], AF.Tanh)
                    ex_t = wpool.tile([128, CSUM], fp16, tag="ex")
                    nc.scalar.activation(ex_t[:], sc_t[:], AF.Exp)

                    for t, (bb, cl) in enumerate(chunks):
                        mt = wpool.tile([128, 65], fp16, tag="mt")
                        nc.vector.tensor_tensor(
                            out=mt[:], in0=Gbs[bb][:, cl, 0:65],
                            in1=ex_t[:, t:t + 1].to_broadcast([128, 65]),
                            op=ALU.mult)
                        nc.tensor.matmul(psA[:, 0:65], lhsT=sts[t][:],
                                         rhs=mt[:], start=(t == 0),
                                         stop=(t == CSUM - 1))

                    dn = wpool.tile([128, 1], f32, tag="dn")
                    nc.vector.tensor_scalar(out=dn[:], in0=psA[:, 64:65],
                                            scalar1=1e-30, scalar2=None,
                                            op0=ALU.max)
                    inv = wpool.tile([128, 1], f32, tag="inv")
                    nc.vector.reciprocal(inv[:], dn[:])
                    nc.vector.tensor_scalar(
                        out=outb[:, wl, :], in0=psA[:, 0:64],
                        scalar1=inv[:], scalar2=(1.0 - EPS),
                        op0=ALU.mult, op1=ALU.mult)
                xeb = ginpool.tile([128, G_WIN, CH], f32, tag="xeb")
                nc.vector.tensor_scalar_mul(
                    xeb[:].rearrange("p a b -> p (a b)"),
                    xw_t[:].rearrange("p a b -> p (a b)"), EPS)
                nc.vector.tensor_tensor(
                    out=outb[:].rearrange("p a b -> p (a b)"),
                    in0=outb[:].rearrange("p a b -> p (a b)"),
                    in1=xeb[:].rearrange("p a b -> p (a b)"), op=ALU.add)
                nc.scalar.dma_start(
                    out=out_d[ts(g, G_WIN * 128), :].rearrange(
                        "(j p) c -> p j c", p=128),
                    in_=outb[:])
    nc.compile()
    return nc


def kernel(x, edge_index, W_att, b_att, W_msg, _trace=False):
    from concourse.bass_utils import run_bass_kernel_spmd

    in_maps, CB, _ = _host_prep(x, edge_index, W_att, b_att, W_msg)
    nc = build_program(CB)
    res = run_bass_kernel_spmd(nc, in_maps, list(range(NCORES)), trace=_trace)
    LAST["res"] = res
    LAST["CB"] = CB
    out = np.concatenate([res.results[c]["out"][:NPC] for c in range(NCORES)],
                         axis=0)
    return np.ascontiguousarray(out, dtype=np.float32)
